# revision 6
# baseline (speedup 1.0000x reference)
"""Multi-head self-attention TRN2 Bass kernel.

Problem: x[2, 2048, 1024], 16 heads x 64 dim, fp32.
Sharding: 8 cores = 2 batches x 4 head-groups (4 heads each).
Each core computes its batch's partial output (its 4 heads through
QKV -> attention -> output projection rows); host sums the 4 partials
per batch and adds bo.

Per-core layout strategy (avoids every attention transpose):
  - x^T loaded straight from HBM via DMA xbar transpose (x cast to bf16
    on host).
  - q^T, k^T [256, 2048] bf16  (head h at partitions (h%2)*64 of tile h//2)
  - V' [2048, 4, 65] bf16  (per head: V columns + a ones column)
  - scores computed TRANSPOSED: S^T[k,q] = k^T.T @ q^T  (bf16 matmuls,
    fp32 PSUM accumulate; 1/sqrt(hd) folded into Wq/bq on host)
  - exp on ACT -> A^T bf16, directly the moving operand of
    out^T[65, q] = V'^T @ A^T ; row 64 = softmax row sums (ones trick).
  - normalize with DVE using gpsimd partition_broadcast of 1/sums.
  - out_proj: per-head K=64 PSUM accumulation with Wo row slices.
"""

import numpy as np

S = 2048          # sequence length per batch
H = 1024          # hidden
G = 256           # head-group width (4 heads x 64)
HD = 65           # V' columns per head (64 + ones)
NHL = 4           # heads per core
N_CORES = 8

_CACHE = {}


def _build():
    if "nc" in _CACHE:
        return _CACHE["nc"]

    import concourse.bass as bass
    import concourse.mybir as mybir
    import concourse.tile as tile
    from concourse import bacc

    f32 = mybir.dt.float32
    bf16 = mybir.dt.bfloat16

    nc = bacc.Bacc("TRN2", target_bir_lowering=False, debug=False,
                   num_devices=N_CORES)

    x_in = nc.dram_tensor("x", [S, H], bf16, kind="ExternalInput")
    wq_in = nc.dram_tensor("wq", [H, G], bf16, kind="ExternalInput")
    wk_in = nc.dram_tensor("wk", [H, G], bf16, kind="ExternalInput")
    wv_in = nc.dram_tensor("wv", [H, G], bf16, kind="ExternalInput")
    bq_in = nc.dram_tensor("bq", [G, 1], f32, kind="ExternalInput")
    bk_in = nc.dram_tensor("bk", [G, 1], f32, kind="ExternalInput")
    bv_in = nc.dram_tensor("bv", [G], f32, kind="ExternalInput")
    wo_in = nc.dram_tensor("wo", [NHL, 64, H], bf16, kind="ExternalInput")
    out_d = nc.dram_tensor("out", [S, H], f32, kind="ExternalOutput")

    with tile.TileContext(nc) as tc:
        with tc.tile_pool(name="persist", bufs=1) as persist:
            qT = persist.tile([128, 2, S], bf16)     # [qd, m, s]
            kT = persist.tile([128, 2, S], bf16)
            vp = persist.tile([128, 16, NHL, HD], bf16)  # [s-part, st, h, col]
            wo_sb = persist.tile([64, NHL, H], bf16)
            bq_sb = persist.tile([128, 2, 1], f32)
            bk_sb = persist.tile([128, 2, 1], f32)
            bv_bc = persist.tile([128, G], f32)

            nc.sync.dma_start(
                out=wo_sb, in_=wo_in.ap().rearrange("h p n -> p h n"))
            nc.sync.dma_start(
                out=bq_sb, in_=bq_in.ap().rearrange("(m p) o -> p m o", p=128))
            nc.sync.dma_start(
                out=bk_sb, in_=bk_in.ap().rearrange("(m p) o -> p m o", p=128))
            # broadcast bv along partitions (stride-0 partition AP)
            bv_ap = bass.AP(tensor=bv_in, offset=0, ap=[[0, 128], [1, G]])
            nc.gpsimd.dma_start(out=bv_bc, in_=bv_ap)

            # ones columns of V'
            nc.gpsimd.memset(vp[:, :, :, 64:65], 1.0)

            # ---------------- Phase A: x^T, QKV projections ----------------
            with (
                tc.tile_pool(name="wqkv", bufs=1) as w_pool,
                tc.tile_pool(name="xT", bufs=1) as xT_pool,
                tc.tile_pool(name="ps_a", bufs=4, space="PSUM") as ps_a,
                tc.tile_pool(name="ps_v", bufs=2, space="PSUM") as ps_v,
            ):
                wq_sb = w_pool.tile([128, 8, G], bf16)
                wk_sb = w_pool.tile([128, 8, G], bf16)
                wv_sb = w_pool.tile([128, 8, G], bf16)
                nc.sync.dma_start(
                    out=wq_sb, in_=wq_in.ap().rearrange("(t p) d -> p t d", p=128))
                nc.sync.dma_start(
                    out=wk_sb, in_=wk_in.ap().rearrange("(t p) d -> p t d", p=128))
                nc.sync.dma_start(
                    out=wv_sb, in_=wv_in.ap().rearrange("(t p) d -> p t d", p=128))

                xT = xT_pool.tile([128, 8, S], bf16)   # [h-part, ht, s]
                # chunk-wise xbar-transpose loads so kT can start early
                for jc in range(4):
                    for ht in range(8):
                        nc.sync.dma_start(
                            out=xT[:, ht, jc * 512:(jc + 1) * 512],
                            in_=x_in.ap()[jc * 512:(jc + 1) * 512,
                                          ht * 128:(ht + 1) * 128],
                            transpose=True)

                # k^T for all chunks first, then q^T, then V: phase B's
                # scores (which need full kT + one qT chunk) unblock early.
                for w_sb, b_sb, dst in ((wk_sb, bk_sb, kT), (wq_sb, bq_sb, qT)):
                    for jc in range(4):
                        sl = slice(jc * 512, (jc + 1) * 512)
                        for m in range(2):
                            ps_q = ps_a.tile([128, 512], f32, tag="qk")
                            for ht in range(8):
                                nc.tensor.matmul(
                                    ps_q,
                                    lhsT=w_sb[:, ht, m * 128:(m + 1) * 128],
                                    rhs=xT[:, ht, sl],
                                    start=(ht == 0), stop=(ht == 7))
                            nc.vector.tensor_scalar_add(
                                dst[:, m, sl], ps_q, b_sb[:, m, :])
                for st in range(16):
                    ps_vt = ps_v.tile([128, G], f32)
                    for ht in range(8):
                        nc.tensor.matmul(
                            ps_vt,
                            lhsT=xT[:, ht, st * 128:(st + 1) * 128],
                            rhs=wv_sb[:, ht, :],
                            start=(ht == 0), stop=(ht == 7))
                    nc.vector.tensor_add(
                        vp[:, st, :, 0:64],
                        ps_vt.rearrange("p (h d) -> p h d", h=NHL),
                        bv_bc.rearrange("p (h d) -> p h d", h=NHL))

            # ---------------- Phase B: attention + out_proj ----------------
            with (
                tc.tile_pool(name="attnT", bufs=2) as at_pool,
                tc.tile_pool(name="outTn", bufs=2) as on_pool,
                tc.tile_pool(name="sums", bufs=4) as sums_pool,
                tc.tile_pool(name="rbc", bufs=3) as rbc_pool,
                tc.tile_pool(name="osb", bufs=2) as osb_pool,
                tc.tile_pool(name="ps_s", bufs=2, space="PSUM") as ps_s_pool,
                tc.tile_pool(name="ps_av", bufs=2, space="PSUM") as ps_av_pool,
                tc.tile_pool(name="ps_op", bufs=2, space="PSUM") as ps_op_pool,
            ):
                for qc in range(4):  # q-chunks of 512
                    qsl = slice(qc * 512, (qc + 1) * 512)
                    outTn = on_pool.tile([64, NHL, 512], bf16)
                    for h in range(4):
                        pb = (h % 2) * 64       # partition base inside qT/kT
                        mt = h // 2             # qT/kT tile index
                        attnT = at_pool.tile([128, 16, 512], bf16)
                        ps_av = ps_av_pool.tile([HD, 512], f32)
                        # per-k-group interleave: scores -> exp -> attn@V so
                        # the PE stays dense while ACT streams exps
                        for kg in range(8):  # groups of 2 k-tiles
                            ps_s = ps_s_pool.tile([128, 2, 512], f32)
                            for i in range(2):
                                kt = kg * 2 + i
                                nc.tensor.matmul(
                                    ps_s[:, i, :],
                                    lhsT=kT[pb:pb + 64, mt,
                                            kt * 128:(kt + 1) * 128],
                                    rhs=qT[pb:pb + 64, mt, qsl],
                                    start=True, stop=True)
                            nc.scalar.activation(
                                out=attnT[:, kg * 2:kg * 2 + 2, :],
                                in_=ps_s,
                                func=mybir.ActivationFunctionType.Exp)
                            for i in range(2):
                                kt = kg * 2 + i
                                nc.tensor.matmul(
                                    ps_av,
                                    lhsT=vp[:, kt, h, :],
                                    rhs=attnT[:, kt, :],
                                    start=(kt == 0), stop=(kt == 15))
                        sums = sums_pool.tile([1, 512], f32)
                        nc.vector.tensor_copy(sums, ps_av[64:65, :])
                        recip = sums_pool.tile([1, 512], f32, tag="recip")
                        nc.vector.reciprocal(recip, sums)
                        rbc = rbc_pool.tile([64, 512], f32)
                        nc.gpsimd.partition_broadcast(rbc, recip)
                        nc.vector.tensor_mul(
                            outTn[:, h, :], ps_av[0:64, :], rbc)
                    # output projection for this q-chunk
                    for qt in range(4):
                        osb = osb_pool.tile([128, H], f32)
                        for ncx in range(2):
                            ps_op = ps_op_pool.tile([128, 512], f32)
                            for h in range(4):
                                nc.tensor.matmul(
                                    ps_op,
                                    lhsT=outTn[:, h, qt * 128:(qt + 1) * 128],
                                    rhs=wo_sb[:, h, ncx * 512:(ncx + 1) * 512],
                                    start=(h == 0), stop=(h == 3))
                            nc.vector.tensor_copy(
                                osb[:, ncx * 512:(ncx + 1) * 512], ps_op)
                        nc.sync.dma_start(
                            out=out_d.ap()[qc * 512 + qt * 128:
                                           qc * 512 + (qt + 1) * 128, :],
                            in_=osb)

    nc.compile()
    _CACHE["nc"] = nc
    return nc


def make_in_maps(x, Wq, bq, Wk, bk, Wv, bv, Wo):
    import ml_dtypes
    bf = ml_dtypes.bfloat16

    x = np.asarray(x, dtype=np.float32)
    Wq = np.asarray(Wq, dtype=np.float32)
    bq = np.asarray(bq, dtype=np.float32)
    Wk = np.asarray(Wk, dtype=np.float32)
    bk = np.asarray(bk, dtype=np.float32)
    Wv = np.asarray(Wv, dtype=np.float32)
    bv = np.asarray(bv, dtype=np.float32)
    Wo = np.asarray(Wo, dtype=np.float32)

    scale = np.float32(1.0 / 8.0)  # 1/sqrt(64)

    in_maps = []
    for core in range(N_CORES):
        b = core // 4
        g = core % 4
        cs = slice(g * G, (g + 1) * G)
        in_maps.append({
            "x": np.ascontiguousarray(x[b]).astype(bf),
            "wq": np.ascontiguousarray(Wq[:, cs] * scale).astype(bf),
            "wk": np.ascontiguousarray(Wk[:, cs]).astype(bf),
            "wv": np.ascontiguousarray(Wv[:, cs]).astype(bf),
            "bq": np.ascontiguousarray((bq[cs] * scale).reshape(G, 1)),
            "bk": np.ascontiguousarray(bk[cs].reshape(G, 1)),
            "bv": np.ascontiguousarray(bv[cs]),
            "wo": np.ascontiguousarray(Wo[cs, :].reshape(NHL, 64, H)).astype(bf),
        })
    return in_maps


def kernel(x, Wq, bq, Wk, bk, Wv, bv, Wo, bo):
    from concourse.bass_utils import run_bass_kernel_spmd

    bo = np.asarray(bo, dtype=np.float32)
    nc = _build()
    in_maps = make_in_maps(x, Wq, bq, Wk, bk, Wv, bv, Wo)
    res = run_bass_kernel_spmd(nc, in_maps, core_ids=list(range(N_CORES)))

    out = np.empty((2, S, H), dtype=np.float32)
    for b in range(2):
        acc = res.results[4 * b]["out"].astype(np.float32)
        for g in range(1, 4):
            acc = acc + res.results[4 * b + g]["out"]
        out[b] = acc + bo
    return out


# revision 10
# speedup vs baseline: 1.1549x; 1.1549x over previous
"""Multi-head self-attention TRN2 Bass kernel.

Problem: x[2, 2048, 1024], 16 heads x 64 dim, fp32.
Sharding: 8 cores = 2 batches x 4 head-groups (4 heads each).
Each core computes its batch's partial output (its 4 heads through
QKV -> attention -> output projection rows); host sums the 4 partials
per batch and adds bo.

Per-core layout strategy (avoids every attention transpose):
  - x^T loaded straight from HBM via DMA xbar transpose (x cast to bf16
    on host).
  - q^T, k^T [256, 2048] bf16  (head h at partitions (h%2)*64 of tile h//2)
  - V' [2048, 4, 65] bf16  (per head: V columns + a ones column)
  - scores computed TRANSPOSED: S^T[k,q] = k^T.T @ q^T  (bf16 matmuls,
    fp32 PSUM accumulate; 1/sqrt(hd) folded into Wq/bq on host)
  - exp on ACT -> A^T bf16, directly the moving operand of
    out^T[65, q] = V'^T @ A^T ; row 64 = softmax row sums (ones trick).
  - normalize with DVE using gpsimd partition_broadcast of 1/sums.
  - out_proj: per-head K=64 PSUM accumulation with Wo row slices.
"""

import numpy as np

S = 2048          # sequence length per batch
H = 1024          # hidden
G = 256           # head-group width (4 heads x 64)
HD = 65           # V' columns per head (64 + ones)
NHL = 4           # heads per core
N_CORES = 8

_CACHE = {}


def _build():
    if "nc" in _CACHE:
        return _CACHE["nc"]

    import concourse.bass as bass
    import concourse.mybir as mybir
    import concourse.tile as tile
    from concourse import bacc

    f32 = mybir.dt.float32
    bf16 = mybir.dt.bfloat16

    nc = bacc.Bacc("TRN2", target_bir_lowering=False, debug=False,
                   num_devices=N_CORES)

    x_in = nc.dram_tensor("x", [S, H], bf16, kind="ExternalInput")
    wq_in = nc.dram_tensor("wq", [H, G], bf16, kind="ExternalInput")
    wk_in = nc.dram_tensor("wk", [H, G], bf16, kind="ExternalInput")
    wv_in = nc.dram_tensor("wv", [H, G], bf16, kind="ExternalInput")
    bq_in = nc.dram_tensor("bq", [G, 1], f32, kind="ExternalInput")
    bk_in = nc.dram_tensor("bk", [G, 1], f32, kind="ExternalInput")
    bv_in = nc.dram_tensor("bv", [G], f32, kind="ExternalInput")
    wo_in = nc.dram_tensor("wo", [NHL, 64, H], bf16, kind="ExternalInput")
    out_d = nc.dram_tensor("out", [S, H], f32, kind="ExternalOutput")

    with tile.TileContext(nc) as tc:
        with tc.tile_pool(name="persist", bufs=1) as persist:
            qT = persist.tile([128, 2, S], bf16)     # [qd, m, s]
            kT = persist.tile([128, 2, S], bf16)
            vp = persist.tile([128, 16, NHL, HD], bf16)  # [s-part, st, h, col]
            bq_sb = persist.tile([128, 2, 1], f32)
            bk_sb = persist.tile([128, 2, 1], f32)
            bv_bc = persist.tile([128, G], f32)

            nc.sync.dma_start(
                out=bq_sb, in_=bq_in.ap().rearrange("(m p) o -> p m o", p=128))
            nc.sync.dma_start(
                out=bk_sb, in_=bk_in.ap().rearrange("(m p) o -> p m o", p=128))
            # broadcast bv along partitions (stride-0 partition AP)
            bv_ap = bass.AP(tensor=bv_in, offset=0, ap=[[0, 128], [1, G]])
            nc.gpsimd.dma_start(out=bv_bc, in_=bv_ap)

            # ones columns of V'
            nc.gpsimd.memset(vp[:, :, :, 64:65], 1.0)

            # ---------------- Phase A: x^T, QKV projections ----------------
            with (
                tc.tile_pool(name="wqkv", bufs=1) as w_pool,
                tc.tile_pool(name="xT", bufs=1) as xT_pool,
                tc.tile_pool(name="ps_a", bufs=4, space="PSUM") as ps_a,
                tc.tile_pool(name="ps_v", bufs=2, space="PSUM") as ps_v,
            ):
                wq_sb = w_pool.tile([128, 8, G], bf16)
                wk_sb = w_pool.tile([128, 8, G], bf16)
                wv_sb = w_pool.tile([128, 8, G], bf16)
                nc.sync.dma_start(
                    out=wq_sb, in_=wq_in.ap().rearrange("(t p) d -> p t d", p=128))
                nc.sync.dma_start(
                    out=wk_sb, in_=wk_in.ap().rearrange("(t p) d -> p t d", p=128))
                nc.sync.dma_start(
                    out=wv_sb, in_=wv_in.ap().rearrange("(t p) d -> p t d", p=128))

                xT = xT_pool.tile([128, 8, S], bf16)   # [h-part, ht, s]
                # chunk-wise xbar-transpose loads so kT can start early
                for jc in range(4):
                    for ht in range(8):
                        nc.sync.dma_start(
                            out=xT[:, ht, jc * 512:(jc + 1) * 512],
                            in_=x_in.ap()[jc * 512:(jc + 1) * 512,
                                          ht * 128:(ht + 1) * 128],
                            transpose=True)

                # k^T for all chunks first, then q^T, then V: phase B's
                # scores (which need full kT + one qT chunk) unblock early.
                for w_sb, b_sb, dst in ((wk_sb, bk_sb, kT), (wq_sb, bq_sb, qT)):
                    for jc in range(4):
                        sl = slice(jc * 512, (jc + 1) * 512)
                        for m in range(2):
                            ps_q = ps_a.tile([128, 512], f32, tag="qk")
                            for ht in range(8):
                                nc.tensor.matmul(
                                    ps_q,
                                    lhsT=w_sb[:, ht, m * 128:(m + 1) * 128],
                                    rhs=xT[:, ht, sl],
                                    start=(ht == 0), stop=(ht == 7))
                            nc.vector.tensor_scalar_add(
                                dst[:, m, sl], ps_q, b_sb[:, m, :])
                for st in range(16):
                    ps_vt = ps_v.tile([128, G], f32)
                    for ht in range(8):
                        nc.tensor.matmul(
                            ps_vt,
                            lhsT=xT[:, ht, st * 128:(st + 1) * 128],
                            rhs=wv_sb[:, ht, :],
                            start=(ht == 0), stop=(ht == 7))
                    nc.vector.tensor_add(
                        vp[:, st, :, 0:64],
                        ps_vt.rearrange("p (h d) -> p h d", h=NHL),
                        bv_bc.rearrange("p (h d) -> p h d", h=NHL))

            # ---------------- Phase B: attention + out_proj ----------------
            # head PAIRS: scores row-tiled (2 concurrent K=64 matmuls),
            # out_proj stacked to K=128; dummy matmuls keep the PE HAM warm
            with (
                tc.tile_pool(name="attnT", bufs=2) as at_pool,
                tc.tile_pool(name="outP", bufs=2) as op_pool,
                tc.tile_pool(name="tmpo", bufs=2) as tmpo_pool,
                tc.tile_pool(name="sums", bufs=4) as sums_pool,
                tc.tile_pool(name="rbc", bufs=3) as rbc_pool,
                tc.tile_pool(name="osb", bufs=2) as osb_pool,
                tc.tile_pool(name="wop", bufs=1) as wop_pool,
                tc.tile_pool(name="ps_s", bufs=2, space="PSUM") as ps_s_pool,
                tc.tile_pool(name="ps_av", bufs=2, space="PSUM") as ps_av_pool,
                tc.tile_pool(name="ps_op", bufs=1, space="PSUM") as ps_op_pool,
            ):
                # Wo as stacked head pairs: [two*64+p, pr, n]
                wo_pr = wop_pool.tile([128, 2, H], bf16)
                nc.sync.dma_start(
                    out=wo_pr,
                    in_=wo_in.ap().rearrange("(pr two) p n -> (two p) pr n", two=2))

                def dummy(n):
                    ps_d = ps_op_pool.tile([128, 512], f32, tag="dummy")
                    nc.tensor.matmul(ps_d[:, 0:n], lhsT=kT[:, 0, 0:128],
                                     rhs=qT[:, 0, 0:n], start=True, stop=True)

                for qc in range(4):  # q-chunks of 512
                    qsl = slice(qc * 512, (qc + 1) * 512)
                    outPs = []
                    for mt in range(2):  # head pair (2mt, 2mt+1)
                        attnT = at_pool.tile([128, 2, 16, 512], bf16)
                        ps_avs = [ps_av_pool.tile([HD, 512], f32, tag="av",
                                                  name=f"av_{qc}_{mt}_{hh}")
                                  for hh in range(2)]
                        for kt in range(16):
                            ps_s = ps_s_pool.tile([128, 2, 512], f32)
                            for hh in range(2):
                                nc.tensor.matmul(
                                    ps_s[:, hh, :],
                                    lhsT=kT[hh * 64:hh * 64 + 64, mt,
                                            kt * 128:(kt + 1) * 128],
                                    rhs=qT[hh * 64:hh * 64 + 64, mt, qsl],
                                    start=True, stop=True)
                            nc.scalar.activation(
                                out=attnT[:, :, kt, :],
                                in_=ps_s,
                                func=mybir.ActivationFunctionType.Exp)
                            for hh in range(2):
                                nc.tensor.matmul(
                                    ps_avs[hh],
                                    lhsT=vp[:, kt, 2 * mt + hh, :],
                                    rhs=attnT[:, hh, kt, :],
                                    start=(kt == 0), stop=(kt == 15))
                            dummy(256)
                        outP = op_pool.tile([128, 512], bf16)
                        for hh in range(2):
                            ps_av = ps_avs[hh]
                            sums = sums_pool.tile([1, 512], f32)
                            nc.vector.tensor_copy(sums, ps_av[64:65, :])
                            recip = sums_pool.tile([1, 512], f32, tag="recip")
                            nc.vector.reciprocal(recip, sums)
                            rbc = rbc_pool.tile([64, 512], f32)
                            nc.gpsimd.partition_broadcast(rbc, recip)
                            if hh == 0:
                                nc.vector.tensor_mul(
                                    outP[0:64, :], ps_av[0:64, :], rbc)
                            else:
                                tmpo = tmpo_pool.tile([64, 512], bf16)
                                nc.vector.tensor_mul(
                                    tmpo, ps_av[0:64, :], rbc)
                                nc.sync.dma_start(out=outP[64:128, :], in_=tmpo)
                        outPs.append(outP)
                        for _ in range(6):
                            dummy(512)
                    # output projection for this q-chunk (K=128 stacked pairs)
                    for qt in range(4):
                        osb = osb_pool.tile([128, H], f32)
                        for ncx in range(2):
                            ps_op = ps_op_pool.tile([128, 512], f32, tag="oproj")
                            for pr in range(2):
                                nc.tensor.matmul(
                                    ps_op,
                                    lhsT=outPs[pr][:, qt * 128:(qt + 1) * 128],
                                    rhs=wo_pr[:, pr, ncx * 512:(ncx + 1) * 512],
                                    start=(pr == 0), stop=(pr == 1))
                            nc.vector.tensor_copy(
                                osb[:, ncx * 512:(ncx + 1) * 512], ps_op)
                        nc.sync.dma_start(
                            out=out_d.ap()[qc * 512 + qt * 128:
                                           qc * 512 + (qt + 1) * 128, :],
                            in_=osb)

    nc.compile()
    _CACHE["nc"] = nc
    return nc


def make_in_maps(x, Wq, bq, Wk, bk, Wv, bv, Wo):
    import ml_dtypes
    bf = ml_dtypes.bfloat16

    x = np.asarray(x, dtype=np.float32)
    Wq = np.asarray(Wq, dtype=np.float32)
    bq = np.asarray(bq, dtype=np.float32)
    Wk = np.asarray(Wk, dtype=np.float32)
    bk = np.asarray(bk, dtype=np.float32)
    Wv = np.asarray(Wv, dtype=np.float32)
    bv = np.asarray(bv, dtype=np.float32)
    Wo = np.asarray(Wo, dtype=np.float32)

    scale = np.float32(1.0 / 8.0)  # 1/sqrt(64)

    in_maps = []
    for core in range(N_CORES):
        b = core // 4
        g = core % 4
        cs = slice(g * G, (g + 1) * G)
        in_maps.append({
            "x": np.ascontiguousarray(x[b]).astype(bf),
            "wq": np.ascontiguousarray(Wq[:, cs] * scale).astype(bf),
            "wk": np.ascontiguousarray(Wk[:, cs]).astype(bf),
            "wv": np.ascontiguousarray(Wv[:, cs]).astype(bf),
            "bq": np.ascontiguousarray((bq[cs] * scale).reshape(G, 1)),
            "bk": np.ascontiguousarray(bk[cs].reshape(G, 1)),
            "bv": np.ascontiguousarray(bv[cs]),
            "wo": np.ascontiguousarray(Wo[cs, :].reshape(NHL, 64, H)).astype(bf),
        })
    return in_maps


def kernel(x, Wq, bq, Wk, bk, Wv, bv, Wo, bo):
    from concourse.bass_utils import run_bass_kernel_spmd

    bo = np.asarray(bo, dtype=np.float32)
    nc = _build()
    in_maps = make_in_maps(x, Wq, bq, Wk, bk, Wv, bv, Wo)
    res = run_bass_kernel_spmd(nc, in_maps, core_ids=list(range(N_CORES)))

    out = np.empty((2, S, H), dtype=np.float32)
    for b in range(2):
        acc = res.results[4 * b]["out"].astype(np.float32)
        for g in range(1, 4):
            acc = acc + res.results[4 * b + g]["out"]
        out[b] = acc + bo
    return out
